# revision 20
# baseline (speedup 1.0000x reference)
"""Trainium2 Bass kernel for AttenComm (V2X feature fusion).

Pipeline per scene b (reference semantics):
  1. Per-pixel ego-vs-neighbor attention scores on two frames -> confidence
  2. threshold + 3x3 maxpool -> binary mask (shared by the 4 neighbors)
  3. per-agent affine warp (bilinear, zeros padding) of masked frame-0
  4. per-pixel 5-agent attention fusion, keep ego row -> (C,H,W)

Sharding: 8 cores = 4 scenes x 2 row-halves.  Each core receives a
pre-sliced 56-row band of both frames (rows offset by half; the host
duplicates the first/last image row so edge clamping needs no kernel
logic), runs scores/mask only over its band, writes a band-relative
channel-last zero-padded masked table "xt" to DRAM, then warps+fuses its
48 output rows using batched dma_gather row-pair gathers with
host-precomputed indices/weights (band-relative, per core).

The threshold test is computed with tanh instead of sigmoid
(conf > 0.5  <=>  w0*tanh(z0/2) + w1*tanh(z1/2) > 0, exactly), which
keeps every activation (Tanh/Exp/Copy) in one ACT table set and avoids
per-chunk activation-table reloads.
"""
import sys
import numpy as np

sys.path.insert(0, '/opt/trn_rl_repo')

B, N, C, H, W = 4, 5, 64, 96, 288
P = H * W                      # 27648 pixels (full frame)
HALF = P // 2                  # 13824 output pixels per core
Wp = W + 2                     # padded row length (zeros cols)

BR = 56                        # conf band rows per core (incl. edge dups)
TR = 53                        # xt table interior rows (= band rows 1..53)
TRP = TR + 2                   # + top/bottom zero-pad rows
TAB = TRP * Wp + 2             # band table pixels (+2 slack for pair reads)
BP = BR * W                    # band pixels = 16128

PC1 = 14                       # pass-1 chunks (4 band rows each)
CH1 = 4 * W                    # 1152 px per pass-1 chunk
PC2 = 7                        # pass-2 chunks
CH2 = 2048                     # px per pass-2 chunk (last one padded)

# per-core band start: real image row of band row 1 (= xt table row 1)
BASE = (0, 43)                 # half 0 -> rows 0..52, half 1 -> rows 43..95


def _chunk_trange(ch):
    """Table-row range [t_lo, t_hi] that pass-2 chunk ch may gather from
    (compile-time, union over both halves; warp reach <= 5 rows)."""
    lo = (ch * CH2) // W
    hi = min(H // 2 - 1, (ch * CH2 + CH2 - 1) // W)
    t_lo = max(0, lo - 4)
    t_hi = min(TRP - 1, hi + 11)
    return t_lo, t_hi

_DEC = None


def _decay_weights():
    global _DEC
    if _DEC is None:
        v = np.array([1.0, 0.5], dtype=np.float32)
        e = np.exp(v - v.max())
        _DEC = (e / e.sum()).astype(np.float32)
    return _DEC


# ---------------------------------------------------------------- device code
_PROG = None


def _build_program():
    import os as _os
    import concourse.bass as bass
    import concourse.bacc as bacc
    import concourse.mybir as mybir
    import concourse.tile as tile
    from concourse import library_config
    from concourse.masks import make_identity
    from contextlib import ExitStack

    F32 = mybir.dt.float32
    I16 = mybir.dt.int16
    ALU = mybir.AluOpType
    ACTF = mybir.ActivationFunctionType
    AX = mybir.AxisListType

    DEC = _decay_weights()
    W0f, W1f = float(DEC[0]), float(DEC[1])

    nc = bacc.Bacc('TRN2', target_bir_lowering=False, debug=False)

    fx = nc.dram_tensor('fx', [2, N, C, BP], F32, kind='ExternalInput')
    f0, f1 = fx[0], fx[1]
    mlp = nc.dram_tensor('mlp', [1, C + 1], F32, kind='ExternalInput')
    idx16 = nc.dram_tensor('idx16', [PC2, 128, N, 2, 128], I16, kind='ExternalInput')
    wts = nc.dram_tensor('wts', [PC2, 128, N, 4, 16], F32, kind='ExternalInput')
    out_l = nc.dram_tensor('out_l', [C, HALF], F32, kind='ExternalOutput')
    xt = nc.dram_tensor('xt', [N, TAB * C], F32)   # padded channel-last tables

    def v(ap, dims, off_elems=0):
        """Build a raw view over ap's tensor with explicit [step,count] dims."""
        return bass.AP(tensor=ap.tensor, offset=ap.offset + off_elems, ap=dims)

    with tile.TileContext(nc) as tc, ExitStack() as ctx:
        const = ctx.enter_context(tc.tile_pool(name='const', bufs=1))
        nc.gpsimd.load_library(library_config.mlp)

        ident = const.tile([128, 128], F32)
        make_identity(nc, ident[:])
        wrow = const.tile([1, C + 1], F32)
        nc.sync.dma_start(wrow[:], mlp[:, :])
        wbrep = const.tile([128, C + 1], F32)
        nc.gpsimd.partition_broadcast(wbrep[:], wrow[:])
        wrep = const.tile([128, C], F32)
        nc.vector.tensor_copy(wrep[:], wbrep[:, 0:C])
        brep2 = const.tile([128, 1], F32)
        nc.vector.tensor_scalar_mul(brep2[:], wbrep[:, C:C + 1], 0.5)
        wcol = const.tile([C, 1], F32)
        nc.sync.dma_start(wcol[:], v(mlp[0], [[1, C], [0, 1]]))
        zz = const.tile([128, 146], F32)
        nc.vector.memset(zz[:], 0.0)

        # ---- zero the pad cells of each agent's table -----------------------
        # head: pad row 0 (Wp px); interior: (col Wp-1 of row r, col 0 of
        # row r+1) pairs for r=1..TR; tail: pad row TRP-1 + 2 slack px.
        for a in range(N):
            base = xt[a]          # AP over flat [TAB*C]
            nc.scalar.dma_start(v(base, [[1, Wp * C]]), zz[:, 0:145])
            nc.scalar.dma_start(
                v(base, [[Wp * C, TR + 1], [1, 2 * C]], off_elems=(Wp - 1) * C),
                zz[0:TR + 1, 0:128])
            tail_px = TAB - (TR + 1) * Wp
            nc.scalar.dma_start(
                v(base, [[1, tail_px * C]], off_elems=(TR + 1) * Wp * C),
                zz[:, 0:146])

        _stage = int(_os.environ.get('K_STAGE', 9))
        JBg = CH2 // 128
        # gather-side pools live across both passes (disjoint from the
        # pass-1 pools) so the xt gathers can overlap pass-1 compute.
        gpool = ctx.enter_context(tc.tile_pool(name='gp', bufs=3))
        idxp = ctx.enter_context(tc.tile_pool(name='idxp', bufs=1))
        idxall = idxp.tile([128, PC2 * N * 2 * 128], I16)
        _ib = N * 2 * 128
        nc.sync.dma_start(
            idxall[:].rearrange('p (q x) -> p q x', q=PC2),
            bass.AP(tensor=idx16, offset=0,
                    ap=[[_ib, 128], [128 * _ib, PC2], [1, _ib]]))
        # ------------------------------------------------------------- pass 1
        ctx1 = ctx.enter_context(ExitStack())
        p1 = ctx1.enter_context(tc.tile_pool(name='p1', bufs=2))
        ldp = ctx1.enter_context(tc.tile_pool(name='ldp', bufs=2))
        stk0p = ctx1.enter_context(tc.tile_pool(name='stk0', bufs=3))
        stk1p = ctx1.enter_context(tc.tile_pool(name='stk1', bufs=2))
        ps1 = ctx1.enter_context(tc.tile_pool(name='ps1', bufs=2, space='PSUM'))
        sc1p = ctx1.enter_context(tc.tile_pool(name='sc1p', bufs=2))
        gridp = ctx1.enter_context(tc.tile_pool(name='gridp', bufs=1))
        mkp = ctx1.enter_context(tc.tile_pool(name='mkp', bufs=2))
        stgp = ctx1.enter_context(tc.tile_pool(name='stgp', bufs=2))

        grid = gridp.tile([BR, W], F32)          # confidence band
        JT = CH1 // 128                          # 9 transposes per agent

        def scores(stk, qd_ps, ch):
            """stk: (128, JT,N,C) pixel-major stack -> sc (128, JT) tanh.
            qd_ps: PSUM tile (128, (N-1)*JT) holding w-dot per neighbor
            (column k*JT+j) computed by PE from the channel-major tiles."""
            neigh = v(stk[:], [[stk[:].ap[0][0], 128], [N * C, JT], [C, N - 1], [1, C]],
                      off_elems=C)
            ego_b = v(stk[:], [[stk[:].ap[0][0], 128], [N * C, JT], [0, N - 1], [1, C]])

            prod = p1.tile([128, JT * (N - 1) * C], F32, tag='prod')
            pr4 = prod[:].rearrange('p (j k c) -> p j k c', j=JT, k=N - 1)
            nc.vector.tensor_tensor(out=pr4, in0=neigh, in1=ego_b, op=ALU.mult)
            sd = p1.tile([128, JT * (N - 1)], F32, tag='sd')
            nc.vector.tensor_reduce(
                out=sd[:].rearrange('p (j k) -> p j k', j=JT),
                in_=pr4, op=ALU.add, axis=AX.X)
            qd = v(qd_ps[:], [[qd_ps[:].ap[0][0], 128], [1, JT], [JT, N - 1]])

            # softmax over the 4 neighbors (innermost)
            sd3 = sd[:].rearrange('p (j k) -> p j k', j=JT)
            nc.vector.tensor_scalar_mul(sd[:], sd[:], 0.125)
            mx = p1.tile([128, JT], F32, tag='mx')
            nc.vector.tensor_reduce(out=mx[:], in_=sd3, op=ALU.max, axis=AX.X)
            mx_b = v(mx[:], [[mx[:].ap[0][0], 128], [1, JT], [0, N - 1]])
            nc.vector.tensor_tensor(out=sd3, in0=sd3, in1=mx_b, op=ALU.subtract)
            nc.scalar.activation(sd[:], sd[:], ACTF.Exp)
            l = p1.tile([128, JT], F32, tag='l')
            nc.vector.tensor_reduce(out=l[:], in_=sd3, op=ALU.add, axis=AX.X)
            nc.vector.reciprocal(l[:], l[:])
            l_b = v(l[:], [[l[:].ap[0][0], 128], [1, JT], [0, N - 1]])
            nc.vector.tensor_tensor(out=sd3, in0=sd3, in1=l_b, op=ALU.mult)
            # z = sum_k attn_k * q_k ; sc = tanh((z + b) / 2)
            nc.vector.tensor_tensor(out=sd3, in0=sd3, in1=qd, op=ALU.mult)
            z = p1.tile([128, JT], F32, tag='z')
            nc.vector.tensor_reduce(out=z[:], in_=sd3, op=ALU.add, axis=AX.X)
            sc = p1.tile([128, JT], F32, tag='sc' + ('A' if ch else 'B'))
            nc.scalar.activation(sc[:], z[:], ACTF.Tanh, bias=brep2[:, 0:1],
                                 scale=0.5)
            return sc

        def transpose_frame(src_tiles, pool, tag):
            """src: list of N (C, CH1) tiles -> (128, JT*N*C) pixel-major stack
            plus PSUM (128, (N-1)*JT) of w-dots (PE matmul per 128-px block).
            partition p, block j <-> pixel 9p+j (within chunk)."""
            stk = pool.tile([128, JT * N * C], F32, tag=tag)
            qd_ps = ps1.tile([128, (N - 1) * JT], F32, tag='qd_ps')
            for a in range(N):
                pt = ps1.tile([128, JT * C], F32, tag='ps_t')
                for j in range(JT):
                    src_v = v(src_tiles[a][:],
                              [[src_tiles[a][:].ap[0][0], C], [JT, 128]], off_elems=j)
                    nc.tensor.transpose(out=pt[:, j * C:(j + 1) * C], in_=src_v,
                                        identity=ident[0:C, 0:C])
                    if a > 0:
                        nc.tensor.matmul(
                            out=qd_ps[:, (a - 1) * JT + j:(a - 1) * JT + j + 1],
                            lhsT=src_v, rhs=wcol[:])
                dst = v(stk[:], [[stk[:].ap[0][0], 128], [N * C, JT], [1, C]],
                        off_elems=a * C)
                nc.scalar.activation(dst, pt[:].rearrange('p (j c) -> p j c', j=JT),
                                     ACTF.Copy)
            return stk, qd_ps

        def mask_flush(cm1, stk0_prev):
            """Mask+write xt table rows [4*cm1, 4*cm1+4) ∩ [1, TR] from band
            chunk cm1 (conf band rows 4c..4c+3 == table rows, stk0 aligned)."""
            t0r = 4 * cm1
            rows = [r for r in range(4) if 1 <= t0r + r <= TR]
            wup = mkp.tile([4, W], F32, tag='wup')
            wmid = mkp.tile([4, W], F32, tag='wmid')
            wdn = mkp.tile([4, W], F32, tag='wdn')
            # conf rows t-1, t, t+1 are always in-band (edge rows duplicated
            # on the host side), so no clamping here.
            nc.sync.dma_start(wup[0:len(rows), :],
                              grid[t0r + rows[0] - 1:t0r + rows[-1], :])
            nc.sync.dma_start(wmid[0:len(rows), :],
                              grid[t0r + rows[0]:t0r + rows[-1] + 1, :])
            nc.sync.dma_start(wdn[0:len(rows), :],
                              grid[t0r + rows[0] + 1:t0r + rows[-1] + 2, :])
            nr = len(rows)
            nc.vector.tensor_tensor(out=wmid[0:nr], in0=wmid[0:nr], in1=wup[0:nr],
                                    op=ALU.max)
            nc.vector.tensor_tensor(out=wmid[0:nr], in0=wmid[0:nr], in1=wdn[0:nr],
                                    op=ALU.max)
            t1 = mkp.tile([4, W], F32, tag='t1')
            nc.vector.tensor_tensor(out=t1[0:nr, 0:W - 1], in0=wmid[0:nr, 0:W - 1],
                                    in1=wmid[0:nr, 1:W], op=ALU.max)
            mf = mkp.tile([4, W], F32, tag='mf')
            nc.vector.tensor_tensor(out=mf[0:nr, 1:W - 1], in0=t1[0:nr, 0:W - 2],
                                    in1=wmid[0:nr, 2:W], op=ALU.max)
            nc.vector.tensor_copy(mf[0:nr, 0:1], t1[0:nr, 0:1])
            nc.vector.tensor_copy(mf[0:nr, W - 1:W], t1[0:nr, W - 2:W - 1])
            nc.vector.tensor_scalar(out=mf[0:nr], in0=mf[0:nr], scalar1=0.0,
                                    scalar2=None, op0=ALU.is_gt)
            # mt: pixel-major mask for the whole 4-row chunk; rows not in
            # [1, TR] get garbage but are never written out.
            mt = mkp.tile([128, JT], F32, tag='mt')
            nc.sync.dma_start(
                v(mt[:], [[mt[:].ap[0][0], 32 * nr], [1, JT]],
                  off_elems=32 * (rows[0] - 0) * mt[:].ap[0][0]),
                mf[0:nr, :])

            # write xt chunk rows; band row r <-> stack partitions 32r..32r+32
            pstep = stk0_prev[:].ap[0][0]
            for a in range(N):
                if a > 0:
                    stg = stgp.tile([128, JT * C], F32, tag='stg')
                    src_in = v(stk0_prev[:], [[pstep, 128], [N * C, JT], [1, C]],
                               off_elems=a * C)
                    mt_b = v(mt[:], [[mt[:].ap[0][0], 128], [1, JT], [0, C]])
                    nc.vector.tensor_tensor(
                        out=stg[:].rearrange('p (j c) -> p j c', j=JT),
                        in0=src_in, in1=mt_b, op=ALU.mult)
                for r in rows:
                    t = t0r + r
                    dst = v(xt[a], [[1, W * C]], off_elems=(t * Wp + 1) * C)
                    if a == 0:
                        src_r = v(stk0_prev[:], [[pstep, 32], [N * C, JT], [1, C]],
                                  off_elems=32 * r * pstep)
                    else:
                        src_r = stg[32 * r:32 * r + 32, :]
                    nc.scalar.dma_start(dst, src_r)

        prev_stk0 = None
        _pc1 = int(_os.environ.get('K_PC1', PC1))
        for cch in range(_pc1):
            s = cch * CH1
            t0s, t1s = [], []
            for a in range(N):
                t0t = ldp.tile([C, CH1], F32, tag=f'ld_{a}')
                nc.sync.dma_start(t0t[:], f0[a][:, s:s + CH1])
                t0s.append(t0t)
            for a in range(N):
                t1t = ldp.tile([C, CH1], F32, tag=f'ld_{a}')
                nc.sync.dma_start(t1t[:], f1[a][:, s:s + CH1])
                t1s.append(t1t)

            stk0, qd0 = transpose_frame(t0s, stk0p, 'stk0')
            stk1, qd1 = transpose_frame(t1s, stk1p, 'stk1')
            if _stage >= 3:
                sc0 = scores(stk0, qd0, 0)
                sc1 = scores(stk1, qd1, 1)
                cf = sc1p.tile([128, JT], F32, tag='cf')
                nc.vector.tensor_scalar_mul(cf[:], sc1[:], W1f)
                nc.vector.scalar_tensor_tensor(out=cf[:], in0=sc0[:], scalar=W0f,
                                               in1=cf[:], op0=ALU.mult, op1=ALU.add)
                nc.sync.dma_start(grid[4 * cch:4 * cch + 4, :], cf[:])

            if _stage >= 4 and cch >= 1:
                mask_flush(cch - 1, prev_stk0)
            prev_stk0 = stk0
        if _pc1 == PC1 and _stage >= 4:
            mask_flush(PC1 - 1, prev_stk0)

        ctx1.close()

        # ------------------------------------------------------------- pass 2
        p2 = ctx.enter_context(tc.tile_pool(name='p2', bufs=2))
        wsp = ctx.enter_context(tc.tile_pool(name='wsp', bufs=2))
        prp = ctx.enter_context(tc.tile_pool(name='prp', bufs=2))
        pr1 = ctx.enter_context(tc.tile_pool(name='pr1', bufs=1))
        ps2 = ctx.enter_context(tc.tile_pool(name='ps2', bufs=4, space='PSUM'))
        obp = ctx.enter_context(tc.tile_pool(name='obp', bufs=2))
        JB = CH2 // 128                          # 16 pixel blocks per chunk

        _pc2 = int(_os.environ.get('K_PC2', PC2))
        if _stage < 9:
            dummy = p2.tile([C, 128], F32, tag='dummy')
            nc.vector.memset(dummy[:], 1.0)
            nc.sync.dma_start(out_l[:, 0:128], dummy[:])
        for ch in range(_pc2 if _stage >= 5 else 0):
            idxt = idxall[:, ch * N * 2 * 128:(ch + 1) * N * 2 * 128]
            wtt = p2.tile([128, N * 4 * 16], F32, tag='wtt')
            nc.sync.dma_start(wtt[:], wts[ch].rearrange('p a q j -> p (a q j)'))

            wstk = wsp.tile([128, JB * N * C], F32, tag='wstk')   # (p, j, k, c)
            for a in range(N):
                gt_t = gpool.tile([128, JB * 128], F32, tag='gt')
                gb_t = gpool.tile([128, JB * 128], F32, tag='gb')
                gt, gb = gt_t[:], gb_t[:]
                t_lo, t_hi = _chunk_trange(ch)
                cnt = min((t_hi - t_lo + 1) * Wp + 2,
                          TAB - t_lo * Wp - 1)
                in_ap = bass.AP(tensor=xt,
                                offset=(a * TAB + t_lo * Wp) * C,
                                ap=[[C, cnt], [1, 2 * C]])
                for t, gdst in ((0, gt), (1, gb)):
                    if _os.environ.get('K_NOGATHER'):
                        nc.vector.memset(gdst, 0.5)
                    else:
                        nc.gpsimd.dma_gather(
                            out_ap=gdst.rearrange('p (b e) -> p b e', e=2 * C),
                            in_ap=in_ap,
                            idxs_ap=idxt[:, (a * 2 + t) * 128:(a * 2 + t + 1) * 128],
                            num_idxs=CH2, num_idxs_reg=CH2,
                            elem_size=2 * C, elem_step=C,
                            single_packet=False)

                def wv(q):
                    off = (a * 4 + q) * 16
                    return v(wtt[:], [[wtt[:].ap[0][0], 128], [1, JB], [0, C]],
                             off_elems=off)

                def gv(g_ap, half):
                    return v(g_ap, [[g_ap.ap[0][0], 128], [2 * C, JB], [1, C]],
                             off_elems=half * C)

                acc = prp.tile([128, JB * C], F32, tag='acc')
                tmp = prp.tile([128, JB * C], F32, tag='tmp')
                a3 = acc[:].rearrange('p (j c) -> p j c', j=JB)
                t3 = tmp[:].rearrange('p (j c) -> p j c', j=JB)
                nc.vector.tensor_tensor(out=a3, in0=gv(gt, 0), in1=wv(0), op=ALU.mult)
                nc.vector.tensor_tensor(out=t3, in0=gv(gt, 1), in1=wv(1), op=ALU.mult)
                nc.vector.tensor_tensor(out=a3, in0=a3, in1=t3, op=ALU.add)
                nc.vector.tensor_tensor(out=t3, in0=gv(gb, 0), in1=wv(2), op=ALU.mult)
                nc.vector.tensor_tensor(out=a3, in0=a3, in1=t3, op=ALU.add)
                nc.vector.tensor_tensor(out=t3, in0=gv(gb, 1), in1=wv(3), op=ALU.mult)
                wdst = v(wstk[:], [[wstk[:].ap[0][0], 128], [N * C, JB], [1, C]],
                         off_elems=a * C)
                nc.vector.tensor_tensor(out=wdst, in0=a3, in1=t3, op=ALU.add)

            # fusion: s_n = sum_c W0*Wn / 8 ; softmax over n ; ctx = sum attn*Wn
            w4 = wstk[:].rearrange('p (j k c) -> p j k c', j=JB, k=N)
            ego_b = v(wstk[:], [[wstk[:].ap[0][0], 128], [N * C, JB], [0, N], [1, C]])
            prod = pr1.tile([128, JB * N * C], F32, tag='prod2')
            pr4 = prod[:].rearrange('p (j k c) -> p j k c', j=JB, k=N)
            nc.vector.tensor_tensor(out=pr4, in0=w4, in1=ego_b, op=ALU.mult)
            sd = p2.tile([128, JB * N], F32, tag='sd2')
            sd3 = sd[:].rearrange('p (j k) -> p j k', j=JB)
            nc.vector.tensor_reduce(out=sd3, in_=pr4, op=ALU.add, axis=AX.X)
            nc.vector.tensor_scalar_mul(sd[:], sd[:], 0.125)
            mx = p2.tile([128, JB], F32, tag='mx2')
            nc.vector.tensor_reduce(out=mx[:], in_=sd3, op=ALU.max, axis=AX.X)
            mx_b = v(mx[:], [[mx[:].ap[0][0], 128], [1, JB], [0, N]])
            nc.vector.tensor_tensor(out=sd3, in0=sd3, in1=mx_b, op=ALU.subtract)
            nc.scalar.activation(sd[:], sd[:], ACTF.Exp)
            l = p2.tile([128, JB], F32, tag='l2')
            nc.vector.tensor_reduce(out=l[:], in_=sd3, op=ALU.add, axis=AX.X)
            nc.vector.reciprocal(l[:], l[:])
            l_b = v(l[:], [[l[:].ap[0][0], 128], [1, JB], [0, N]])
            nc.vector.tensor_tensor(out=sd3, in0=sd3, in1=l_b, op=ALU.mult)
            at_b = v(sd[:], [[sd[:].ap[0][0], 128], [N, JB], [1, N], [0, C]])
            nc.vector.tensor_tensor(out=pr4, in0=w4, in1=at_b, op=ALU.mult)

            def kv(k):
                return v(prod[:], [[prod[:].ap[0][0], 128], [N * C, JB], [1, C]],
                         off_elems=k * C)
            sA = prp.tile([128, JB * C], F32, tag='acc')
            sB = prp.tile([128, JB * C], F32, tag='tmp')
            sA3 = sA[:].rearrange('p (j c) -> p j c', j=JB)
            sB3 = sB[:].rearrange('p (j c) -> p j c', j=JB)
            nc.vector.tensor_tensor(out=sA3, in0=kv(0), in1=kv(1), op=ALU.add)
            nc.vector.tensor_tensor(out=sB3, in0=kv(2), in1=kv(3), op=ALU.add)
            nc.vector.tensor_tensor(out=sA3, in0=sA3, in1=sB3, op=ALU.add)
            ctxt = p2.tile([128, JB * C], F32, tag='ctxt')
            c3 = ctxt[:].rearrange('p (j c) -> p j c', j=JB)
            nc.vector.tensor_tensor(out=c3, in0=sA3, in1=kv(4), op=ALU.add)

            # transpose back to (C, px) and store
            obuf = obp.tile([C, CH2], F32, tag='obuf')
            for jg in range(JB // 4):
                pt = ps2.tile([C, 512], F32, tag='ps_o')
                for j4 in range(4):
                    j = jg * 4 + j4
                    nc.tensor.transpose(out=pt[:, j4 * 128:(j4 + 1) * 128],
                                        in_=ctxt[:, j * C:(j + 1) * C],
                                        identity=ident[:])
                nc.vector.tensor_copy(obuf[:, jg * 512:(jg + 1) * 512], pt[:])
            s2 = ch * CH2
            npx = min(CH2, HALF - s2)
            nc.scalar.dma_start(out_l[:, s2:s2 + npx], obuf[:, 0:npx])

    nc.compile()
    return nc


# ------------------------------------------------------------------- host side
def _affine_params(pairwise_t_matrix):
    """theta per (b, n): normalized 2x3 affine (reference lines 64-70)."""
    pm = np.asarray(pairwise_t_matrix, dtype=np.float32)
    t = pm[:, :, :, 0:2][:, :, :, :, [0, 1, 3]].copy()        # (B,L,L,2,3)
    t[..., 0, 1] = t[..., 0, 1] * np.float32(H / W)
    t[..., 1, 0] = t[..., 1, 0] * np.float32(W / H)
    t[..., 0, 2] = t[..., 0, 2] / np.float32(4 * 0.4 * W) * np.float32(2)
    t[..., 1, 2] = t[..., 1, 2] / np.float32(4 * 0.4 * H) * np.float32(2)
    return t[:, 0]                                             # (B,N,2,3)


def _warp_tables(theta_bn, base):
    """Per (b,n): int32 idx_top/idx_bot into the band table + 4 weights,
    for this core's 48 output rows (rows [half*48, half*48+48))."""
    half0 = base == 0
    r0 = 0 if half0 else H // 2
    xs = np.linspace(-1.0, 1.0, W, dtype=np.float32)
    ys = np.linspace(-1.0, 1.0, H, dtype=np.float32)[r0:r0 + H // 2]
    gy, gx = np.meshgrid(ys, xs, indexing='ij')
    npx = HALF
    base3 = np.stack([gx.ravel(), gy.ravel(), np.ones(npx, np.float32)], 0)
    g = (theta_bn.astype(np.float32) @ base3).astype(np.float32)          # (2,P/2)
    px = (g[0] + np.float32(1.0)) * np.float32(0.5) * np.float32(W - 1)
    py = (g[1] + np.float32(1.0)) * np.float32(0.5) * np.float32(H - 1)
    x0 = np.floor(px)
    y0 = np.floor(py)
    wx = px - x0
    wy = py - y0
    vx = ((x0 >= -1) & (x0 <= W - 1)).astype(np.float32)
    x0c = np.clip(x0, -1, W - 1).astype(np.int32)
    y0c = np.clip(y0, -1, H).astype(np.int32)
    y1c = np.clip(y0 + 1, -1, H).astype(np.int32)

    def trow(y):
        # real image row -> band table row; out-of-image rows -> zero pads
        t = y - (base - 1)
        t = np.where(y < 0, 0, t)
        t = np.where(y > H - 1, TRP - 1, t)
        return t

    idx_t = trow(y0c) * Wp + (x0c + 1)
    idx_b = trow(y1c) * Wp + (x0c + 1)
    w00 = (1 - wy) * (1 - wx) * vx
    w01 = (1 - wy) * wx * vx
    w10 = wy * (1 - wx) * vx
    w11 = wy * wx * vx
    return idx_t, idx_b, np.stack([w00, w01, w10, w11]).astype(np.float32)


def _pack_core_tables(theta_b, half):
    """idx16 (PC2,128,N,2,128) int16 + wts (PC2,128,N,4,16) f32 for one core."""
    idx16 = np.zeros((PC2, 128, N, 2, 128), np.int16)
    wq = np.zeros((PC2, 128, N, 4, 16), np.float32)
    for a in range(N):
        it, ib, w4 = _warp_tables(theta_b[a], BASE[half])
        for ch in range(PC2):
            s = ch * CH2
            npx = min(CH2, HALF - s)
            iv_t = np.zeros(CH2, np.int32)
            iv_b = np.zeros(CH2, np.int32)
            wv = np.zeros((4, CH2), np.float32)
            t_lo, _ = _chunk_trange(ch)
            iv_t[:npx] = it[s:s + npx] - t_lo * Wp
            iv_b[:npx] = ib[s:s + npx] - t_lo * Wp
            wv[:, :npx] = w4[:, s:s + npx]
            # idx buffer: index i -> [i%16 (replicated +16r), i//16]
            bt = iv_t.reshape(128, 16).T.astype(np.int16)      # (16,128)
            bb = iv_b.reshape(128, 16).T.astype(np.int16)
            idx16[ch, :, a, 0, :] = np.tile(bt, (8, 1))
            idx16[ch, :, a, 1, :] = np.tile(bb, (8, 1))
            # weights: pixel i = j*128 + p -> [p, q, j]
            wq[ch, :, a, :, :] = wv.reshape(4, 16, 128).transpose(2, 0, 1)
    return idx16, wq


_ROWSEL = None


def _row_selections():
    """Per half: the 56 real image rows shipped as the conf band (with edge
    rows duplicated so band rows 0..55 = table rows 0..55 need no clamping)."""
    global _ROWSEL
    if _ROWSEL is None:
        sel0 = np.clip(np.arange(BR) - 1, 0, H - 1)        # [-1,0..54] -> dup row 0
        sel1 = np.clip(np.arange(BR) + 42, 0, H - 1)       # [42..97] -> dup row 95
        _ROWSEL = (sel0.astype(np.intp), sel1.astype(np.intp))
    return _ROWSEL


def _in_maps_for_cores(inputs):
    """Build the per-core input dicts from the full problem inputs."""
    x0 = np.asarray(inputs['x0'], np.float32).reshape(B, N, C, H, W)
    x1 = np.asarray(inputs['x1'], np.float32).reshape(B, N, C, H, W)
    theta = _affine_params(inputs['pairwise_t_matrix'])
    mw = np.asarray(inputs['mlp_w'], np.float32).reshape(1, C)
    mb = np.asarray(inputs['mlp_b'], np.float32).reshape(1, 1)
    mlp = np.concatenate([mw, mb], axis=1)                     # (1, C+1)
    sels = _row_selections()

    in_maps = []
    for core in range(8):
        b, half = core // 2, core % 2
        idx16, wq = _pack_core_tables(theta[b], half)
        sel = sels[half]
        fx = np.stack([
            x0[b][:, :, sel, :].reshape(N, C, BP),
            x1[b][:, :, sel, :].reshape(N, C, BP)])
        in_maps.append({
            'fx': np.ascontiguousarray(fx), 'mlp': mlp,
            'idx16': idx16, 'wts': wq,
        })
    return in_maps


def kernel(x0, x1, pairwise_t_matrix, mlp_w, mlp_b, record_len=None, _bench=None):
    global _PROG
    from concourse.bass_utils import run_bass_kernel_spmd

    if _PROG is None:
        _PROG = _build_program()
    nc = _PROG

    in_maps = _in_maps_for_cores(dict(
        x0=x0, x1=x1, pairwise_t_matrix=pairwise_t_matrix,
        mlp_w=mlp_w, mlp_b=mlp_b))

    res = run_bass_kernel_spmd(nc, in_maps, list(range(8)))
    if _bench is not None:
        _bench.append(res)

    out = np.empty((B, C, P), np.float32)
    for core in range(8):
        b, half = core // 2, core % 2
        out[b][:, half * HALF:(half + 1) * HALF] = res.results[core]['out_l']
    return out.reshape(B, C, H, W)
